# revision 36
# baseline (speedup 1.0000x reference)
"""nn_MergeWindows — Trainium2 Bass kernel (8 NeuronCores, SPMD over image rows).

The reference's output is out[b, c, y, x] = 1.0 iff remap[argmax_d masks[b, d,
y, x]] == c, where remap: [32]->[32] is the channel-merge map decided by the
sequential scan.  remap depends only on tiny metadata (boundary-strip argmaxes
+ slot-feature cosine sims) and is computed on the host in microseconds.

Device pipeline per [128 rows, 32 ch, G cols] tile, everything on the DVE:
  1. 5-level contiguous max-tree over the channel dim -> mx [128, G]
  2. win = is_equal(in, mx_broadcast)  -> one-hot over channels, bf16
  3. merge fixups: win[:, keep] += win[:, rem]; win[:, rem] = 0
     (a dozen tiny [128, G] ops, baked from remap)
  4. DMA the bf16 one-hot out; the host casts to f32 (0/1 exact in bf16).

Tied maxima (in[c] == in[d] == mx) make is_equal fire twice where argmax picks
the first channel.  Those pixels are exactly where the output's channel-sum is
not 1.0, so the host detects them from the assembled output and patches them
from the input directly (a handful of pixels on real data).
"""

import json

import numpy as np

N_WINDOWS = 4
WIN_H = WIN_W = 512
IMG_H = IMG_W = 1024
C = 32
MPW = C // N_WINDOWS
SLOT_DIM = 64
SIM_THRESH = 0.1

N_CORES = 8
ROWS_PER_CORE = IMG_H // N_CORES  # 128
G = 512                           # column-tile width
NTILES = IMG_W // G

_cache = {}


# --------------------------------------------------------------------------
# host-side merge decision (mirrors reference._merge_windows metadata math)
# --------------------------------------------------------------------------
def _compute_remap(masks, slot_features, pl, pt):
    B, Ch, H, W = masks.shape
    mpw = Ch // N_WINDOWS
    ranges = [(i * mpw, (i + 1) * mpw) for i in range(N_WINDOWS)]

    adjacency = []
    for i in range(N_WINDOWS):
        for j in range(i + 1, N_WINDOWS):
            if pt[i] == pt[j] and abs(pl[i] - pl[j]) == WIN_W:
                adjacency.append((i, j, True) if pl[i] < pl[j] else (j, i, True))
            if pl[i] == pl[j] and abs(pt[i] - pt[j]) == WIN_H:
                adjacency.append((i, j, False) if pt[i] < pt[j] else (j, i, False))

    edge_l = np.zeros(Ch, bool)
    edge_r = np.zeros(Ch, bool)
    edge_t = np.zeros(Ch, bool)
    edge_b = np.zeros(Ch, bool)
    m0 = masks[0]
    for wi, (s, e) in enumerate(ranges):
        ys, ye = max(pt[wi], 0), min(pt[wi] + WIN_H, H)
        xs, xe = max(pl[wi], 0), min(pl[wi] + WIN_W, W)
        if ys >= ye or xs >= xe:
            continue
        ids_l = np.argmax(m0[:, ys:ye, xs], axis=0)
        ids_r = np.argmax(m0[:, ys:ye, xe - 1], axis=0)
        ids_t = np.argmax(m0[:, ys, xs:xe], axis=0)
        ids_b = np.argmax(m0[:, ye - 1, xs:xe], axis=0)
        for k in range(s, e):
            edge_l[k] = np.any(ids_l == k)
            edge_r[k] = np.any(ids_r == k)
            edge_t[k] = np.any(ids_t == k)
            edge_b[k] = np.any(ids_b == k)

    ci_l, cj_l, wi_l, wj_l, hz_l = [], [], [], [], []
    for wi, wj, horiz in adjacency:
        si, ei = ranges[wi]
        sj, ej = ranges[wj]
        for ci in range(si + 1, ei):
            for cj in range(sj + 1, ej):
                ci_l.append(ci)
                cj_l.append(cj)
                wi_l.append(wi)
                wj_l.append(wj)
                hz_l.append(horiz)

    target = np.arange(Ch)
    if not ci_l:
        return target

    sf = np.asarray(slot_features, np.float32)
    sf_n = sf / (np.linalg.norm(sf, axis=-1, keepdims=True) + np.float32(1e-8))
    ci_a = np.array(ci_l)
    cj_a = np.array(cj_l)
    rel_i = ci_a % mpw - 1
    rel_j = cj_a % mpw - 1
    fi = sf_n[np.array(wi_l), rel_i]
    fj = sf_n[np.array(wj_l), rel_j]
    sims = np.sum(fi * fj, axis=-1)
    hz = np.array(hz_l)
    edge_ok = np.where(hz, edge_r[ci_a] & edge_l[cj_a], edge_b[ci_a] & edge_t[cj_a])
    passing = edge_ok & (sims > np.float32(SIM_THRESH))

    merged = np.zeros(Ch, bool)
    for ci, cj, ok in zip(ci_l, cj_l, passing):
        if ok and not merged[ci] and not merged[cj]:
            keep, rem = min(ci, cj), max(ci, cj)
            target[target == rem] = keep
            merged[rem] = True
    return target


def _plan(remap):
    """Derive the device-op plan from remap.

    Returns (eff, keeps, groups) where eff is the sorted list of effective
    output channels (remap[c] == c; the rest are all-zero planes the host
    fills), keeps those eff channels that absorb merged channels, and
    groups[k] the full source list (k plus its merged channels).
    """
    eff = [c for c in range(C) if remap[c] == c]
    groups = {}
    for r in range(C):
        k = int(remap[r])
        if k != r:
            groups.setdefault(k, [k]).append(r)
    keeps = sorted(groups)
    return eff, keeps, groups


# --------------------------------------------------------------------------
# wait-split post-pass: the pinned neuronxcc allows only ONE sync wait per
# instruction; hoist extras onto preceding same-engine EventSemaphore insts.
# --------------------------------------------------------------------------
def _split_excess_waits(bir_json_bytes, limit=1):
    j = json.loads(bir_json_bytes)
    counter = [0]
    for fn in j.get("functions", []):
        for bb in fn.get("blocks", []):
            new_insts = []
            for inst in bb.get("instructions", []):
                si = inst.get("sync_info") or {}
                waits = si.get("on_wait") or []
                if len(waits) > limit:
                    extra = waits[: len(waits) - limit]
                    si["on_wait"] = waits[len(waits) - limit:]
                    inst["sync_info"] = si
                    for i in range(0, len(extra), limit):
                        counter[0] += 1
                        new_insts.append({
                            "engine": inst["engine"],
                            "ins": [],
                            "name": f"{inst['name']}_hoistw{counter[0]}",
                            "opcode": "EventSemaphore",
                            "outs": [],
                            "sync_info": {"on_update": [],
                                          "on_wait": extra[i: i + limit]},
                        })
                new_insts.append(inst)
            bb["instructions"] = new_insts
    return json.dumps(j).encode()


def _build_program(remap):
    key = tuple(int(v) for v in remap)
    if key in _cache:
        return _cache[key]

    import concourse.bass as bass
    import concourse.tile as tile
    from concourse import mybir

    f32 = mybir.dt.float32
    bf16 = mybir.dt.bfloat16
    eff, keeps, groups = _plan(remap)
    NE = len(eff)
    NK = len(keeps)
    kidx = {k: i for i, k in enumerate(keeps)}

    nc = bass.Bass()
    masks_in = nc.dram_tensor("masks", [C, ROWS_PER_CORE, IMG_W], f32,
                              kind="ExternalInput")
    out_dram = nc.dram_tensor("out", [NE, ROWS_PER_CORE, IMG_W], bf16,
                              kind="ExternalOutput")

    with tile.TileContext(nc) as tc:
        with tc.tile_pool(name="main", bufs=NTILES if NTILES > 1 else 1) as pool:
            inp = outp = tree = pool
            plains = [c for c in eff if c not in groups]

            # one-hot source runs: (j0, src_tile_name, c0, n)
            runs = []
            for j, c in enumerate(eff):
                if c in groups:
                    src, c0 = "gs", kidx[c]
                else:
                    src, c0 = "in", c
                if runs and runs[-1][1] == src and \
                        runs[-1][2] + runs[-1][3] == c0 and \
                        runs[-1][0] + runs[-1][3] == j:
                    runs[-1][3] += 1
                else:
                    runs.append([j, src, c0, 1])
            for t in range(NTILES):
                sl = slice(G * t, G * (t + 1))
                in_tile = inp.tile([128, C, G], f32, tag="in_tile")
                # channel-group split so the max-folds can start before the
                # whole tile lands; alternate dispatch queues
                cgroups = ([(0, 4), (4, 8), (8, 16), (16, 24), (24, 32)]
                           if t == 0 else [(0, 8), (8, 16), (16, 24), (24, 32)])
                for c0, c1 in cgroups:
                    nc.sync.dma_start(
                        in_tile[:, c0:c1, :],
                        masks_in[c0:c1, :, sl].rearrange(
                            "d p g -> p d g"))

                # global max via per-keep group maxes + plain folds, emitted
                # in channel-availability order.  gs[0:NK] stays intact for
                # the one-hot compare.
                gs = tree.tile([128, NK + 1, G], f32, tag="gs")
                acc = gs[:, NK, :]
                MAX = mybir.AluOpType.max

                def TT(out, in0, in1, op):
                    # scalar_tensor_tensor: same math via InstTensorScalarPtr,
                    # which supports the DVE 2x_2p perf mode (all-SBUF operands)
                    nc.vector.scalar_tensor_tensor(
                        out=out, in0=in0, scalar=0.0, in1=in1,
                        op0=mybir.AluOpType.bypass, op1=op)
                prog = {k: 0 for k in keeps}
                acc_items = []   # APs not yet folded into acc
                dup_done = set()
                emitted = 0

                def fold_into_acc():
                    nonlocal emitted
                    while acc_items and (len(acc_items) >= 2 or emitted):
                        if not emitted:
                            a, b = acc_items.pop(0), acc_items.pop(0)
                            TT(out=acc, in0=a, in1=b, op=MAX)
                        else:
                            TT(out=acc, in0=acc, in1=acc_items.pop(0), op=MAX)
                        emitted += 1

                for _, b in cgroups:
                    for k in keeps:
                        srcs, i = groups[k], kidx[k]
                        while prog[k] < len(srcs):
                            jn = prog[k]
                            if jn == 0:
                                if len(srcs) < 2 or srcs[1] >= b:
                                    break
                                TT(out=gs[:, i, :], in0=in_tile[:, srcs[0], :],
                                   in1=in_tile[:, srcs[1], :], op=MAX)
                                prog[k] = 2
                            elif srcs[jn] < b:
                                TT(out=gs[:, i, :], in0=gs[:, i, :],
                                   in1=in_tile[:, srcs[jn], :], op=MAX)
                                prog[k] += 1
                            else:
                                break
                        if prog[k] == len(srcs) and prog[k] > 0:
                            acc_items.append(gs[:, i, :])
                            prog[k] += 1  # mark folded
                    newly = [c for c in plains if c < b]
                    for c in newly:
                        acc_items.append(in_tile[:, c, :])
                        plains = [p for p in plains if p != c]
                    if t == 0 and b <= 8:
                        # fill DMA-starved ramp: fold available keep leads
                        # now; max() tolerates the duplicate when the group
                        # max folds in later
                        for k in keeps:
                            srcs = groups[k]
                            if prog[k] == 0 and srcs[0] < b and \
                                    k not in dup_done:
                                acc_items.append(in_tile[:, srcs[0], :])
                                dup_done.add(k)
                    fold_into_acc()
                plains = [c for c in eff if c not in groups]  # reset for next t

                # one-hot over effective channels; two halves, each DMA'd as
                # soon as its runs are done.  The global max lives in acc
                # (gs[:, NK, :]); mx tile is unused.
                mx_ap = acc
                out_tile = outp.tile([128, NE, G], bf16, tag="out_tile")

                def emit_half(hruns, j0, j1):
                    for r0, src, c0, n in hruns:
                        mx_b = bass.AP(
                            tensor=mx_ap.tensor, offset=mx_ap.offset,
                            ap=[mx_ap.ap[0], [0, n], mx_ap.ap[-1]])
                        src_ap = (gs if src == "gs" else in_tile)[:, c0:c0 + n, :]
                        TT(out=out_tile[:, r0:r0 + n, :], in0=src_ap,
                           in1=mx_b, op=mybir.AluOpType.is_equal)
                    nc.scalar.dma_start(
                        out_dram[j0:j1, :, sl].rearrange("c p g -> p c g"),
                        out_tile[:, j0:j1, :])

                # emit one-hot + out-DMA in ~NE/4-channel pieces so the
                # out stream overlaps the compares and the tail is short
                target = max(2, (NE + 3) // 4)
                bounds, nxt = [], target
                for j0b, _, _, nb in runs:
                    if j0b + nb >= nxt and j0b + nb < NE:
                        bounds.append(j0b + nb)
                        nxt = j0b + nb + target
                pieces, lo = [], 0
                for bd in bounds + [NE]:
                    pieces.append((lo, bd))
                    lo = bd
                for lo, hi in pieces:
                    emit_half([r for r in runs if lo <= r[0] < hi], lo, hi)

    orig = nc.to_json_bytes
    nc.to_json_bytes = lambda: _split_excess_waits(orig())
    _cache[key] = nc
    return nc


def kernel(masks, slot_features, pad_left, pad_top):
    from concourse.bass_utils import run_bass_kernel_spmd

    masks = np.asarray(masks, np.float32)
    slot_features = np.asarray(slot_features, np.float32)
    pl = [int(v) for v in np.asarray(pad_left)]
    pt = [int(v) for v in np.asarray(pad_top)]

    remap = _compute_remap(masks, slot_features, pl, pt)
    nc = _build_program(remap)

    in_maps = []
    for i in range(N_CORES):
        slab = np.ascontiguousarray(
            masks[0, :, i * ROWS_PER_CORE:(i + 1) * ROWS_PER_CORE, :])
        in_maps.append({"masks": slab})

    res = run_bass_kernel_spmd(nc, in_maps, core_ids=list(range(N_CORES)))

    eff, _, _ = _plan(remap)
    out = np.zeros((1, C, IMG_H, IMG_W), np.float32)
    for i, r in enumerate(res.results):
        out[0, eff, i * ROWS_PER_CORE:(i + 1) * ROWS_PER_CORE, :] = (
            np.asarray(r["out"]).astype(np.float32))

    # patch tied-max pixels (channel-sum != 1) from the input directly
    s = out[0].sum(axis=0)
    ys, xs = np.nonzero(s != 1.0)
    for y, x in zip(ys, xs):
        d = int(np.argmax(masks[0, :, y, x]))
        out[0, :, y, x] = 0.0
        out[0, remap[d], y, x] = 1.0
    return out


# revision 63
# speedup vs baseline: 1.1190x; 1.1190x over previous
"""nn_MergeWindows — Trainium2 Bass kernel (8 NeuronCores, SPMD over image rows).

The reference's output is out[b, c, y, x] = 1.0 iff remap[argmax_d masks[b, d,
y, x]] == c, where remap: [32]->[32] is the channel-merge map decided by the
sequential scan.  remap depends only on tiny metadata (boundary-strip argmaxes
+ slot-feature cosine sims) and is computed on the host in microseconds.

Merged-away channels (remap[c] != c) are all-zero output planes: the host
fills them via np.zeros and the device only computes/transfers the NE
effective channels.  For a keep channel the one-hot is is_equal(group_max,
global_max) — a group member equals the global max iff one of them is it —
which absorbs the channel merge into the max computation with no fixup ops.

Device pipeline per [128 rows, 32 ch, G cols] tile, everything on the DVE:
  1. per-keep group maxes over its merged sources -> gs[128, NK, G]
  2. global max = fold(plain channels, group maxes) -> acc [128, G],
     ops emitted in channel-group-DMA availability order
  3. out[j] = is_equal(src_j, acc_b), src = gs for keeps else in;  bf16
  4. piecewise out-DMA (overlaps the compares); host casts to f32
     (0/1 exact in bf16).

DVE cost is the floor: ~(C-1+NE)*W cycles/core at ~1 elem/partition/cycle;
in-DMA (16 MiB f32/core) runs at the ~360 GB/s per-core HBM cap alongside.

Tied maxima (in[c] == in[d] == mx) make is_equal fire twice where argmax picks
the first channel.  Those pixels are exactly where the output's channel-sum is
not 1.0, so the host detects them from the assembled output and patches them
from the input directly (a handful of pixels on real data).
"""

import json

import numpy as np

N_WINDOWS = 4
WIN_H = WIN_W = 512
IMG_H = IMG_W = 1024
C = 32
MPW = C // N_WINDOWS
SLOT_DIM = 64
SIM_THRESH = 0.1

N_CORES = 8
ROWS_PER_CORE = IMG_H // N_CORES  # 128
G = 512                           # column-tile width
NTILES = IMG_W // G

_cache = {}


# --------------------------------------------------------------------------
# host-side merge decision (mirrors reference._merge_windows metadata math)
# --------------------------------------------------------------------------
def _compute_remap(masks, slot_features, pl, pt):
    B, Ch, H, W = masks.shape
    mpw = Ch // N_WINDOWS
    ranges = [(i * mpw, (i + 1) * mpw) for i in range(N_WINDOWS)]

    adjacency = []
    for i in range(N_WINDOWS):
        for j in range(i + 1, N_WINDOWS):
            if pt[i] == pt[j] and abs(pl[i] - pl[j]) == WIN_W:
                adjacency.append((i, j, True) if pl[i] < pl[j] else (j, i, True))
            if pl[i] == pl[j] and abs(pt[i] - pt[j]) == WIN_H:
                adjacency.append((i, j, False) if pt[i] < pt[j] else (j, i, False))

    edge_l = np.zeros(Ch, bool)
    edge_r = np.zeros(Ch, bool)
    edge_t = np.zeros(Ch, bool)
    edge_b = np.zeros(Ch, bool)
    m0 = masks[0]
    for wi, (s, e) in enumerate(ranges):
        ys, ye = max(pt[wi], 0), min(pt[wi] + WIN_H, H)
        xs, xe = max(pl[wi], 0), min(pl[wi] + WIN_W, W)
        if ys >= ye or xs >= xe:
            continue
        ids_l = np.argmax(m0[:, ys:ye, xs], axis=0)
        ids_r = np.argmax(m0[:, ys:ye, xe - 1], axis=0)
        ids_t = np.argmax(m0[:, ys, xs:xe], axis=0)
        ids_b = np.argmax(m0[:, ye - 1, xs:xe], axis=0)
        for k in range(s, e):
            edge_l[k] = np.any(ids_l == k)
            edge_r[k] = np.any(ids_r == k)
            edge_t[k] = np.any(ids_t == k)
            edge_b[k] = np.any(ids_b == k)

    ci_l, cj_l, wi_l, wj_l, hz_l = [], [], [], [], []
    for wi, wj, horiz in adjacency:
        si, ei = ranges[wi]
        sj, ej = ranges[wj]
        for ci in range(si + 1, ei):
            for cj in range(sj + 1, ej):
                ci_l.append(ci)
                cj_l.append(cj)
                wi_l.append(wi)
                wj_l.append(wj)
                hz_l.append(horiz)

    target = np.arange(Ch)
    if not ci_l:
        return target

    sf = np.asarray(slot_features, np.float32)
    sf_n = sf / (np.linalg.norm(sf, axis=-1, keepdims=True) + np.float32(1e-8))
    ci_a = np.array(ci_l)
    cj_a = np.array(cj_l)
    rel_i = ci_a % mpw - 1
    rel_j = cj_a % mpw - 1
    fi = sf_n[np.array(wi_l), rel_i]
    fj = sf_n[np.array(wj_l), rel_j]
    sims = np.sum(fi * fj, axis=-1)
    hz = np.array(hz_l)
    edge_ok = np.where(hz, edge_r[ci_a] & edge_l[cj_a], edge_b[ci_a] & edge_t[cj_a])
    passing = edge_ok & (sims > np.float32(SIM_THRESH))

    merged = np.zeros(Ch, bool)
    for ci, cj, ok in zip(ci_l, cj_l, passing):
        if ok and not merged[ci] and not merged[cj]:
            keep, rem = min(ci, cj), max(ci, cj)
            target[target == rem] = keep
            merged[rem] = True
    return target


def _plan(remap):
    """Derive the device-op plan from remap.

    Returns (eff, keeps, groups) where eff is the sorted list of effective
    output channels (remap[c] == c; the rest are all-zero planes the host
    fills), keeps those eff channels that absorb merged channels, and
    groups[k] the full source list (k plus its merged channels).
    """
    eff = [c for c in range(C) if remap[c] == c]
    groups = {}
    for r in range(C):
        k = int(remap[r])
        if k != r:
            groups.setdefault(k, [k]).append(r)
    keeps = sorted(groups)
    return eff, keeps, groups


# --------------------------------------------------------------------------
# wait-split post-pass: the pinned neuronxcc allows only ONE sync wait per
# instruction; hoist extras onto preceding same-engine EventSemaphore insts.
# --------------------------------------------------------------------------
def _split_excess_waits(bir_json_bytes, limit=1):
    j = json.loads(bir_json_bytes)
    counter = [0]
    for fn in j.get("functions", []):
        for bb in fn.get("blocks", []):
            new_insts = []
            for inst in bb.get("instructions", []):
                si = inst.get("sync_info") or {}
                waits = si.get("on_wait") or []
                if len(waits) > limit:
                    extra = waits[: len(waits) - limit]
                    si["on_wait"] = waits[len(waits) - limit:]
                    inst["sync_info"] = si
                    for i in range(0, len(extra), limit):
                        counter[0] += 1
                        new_insts.append({
                            "engine": inst["engine"],
                            "ins": [],
                            "name": f"{inst['name']}_hoistw{counter[0]}",
                            "opcode": "EventSemaphore",
                            "outs": [],
                            "sync_info": {"on_update": [],
                                          "on_wait": extra[i: i + limit]},
                        })
                new_insts.append(inst)
            bb["instructions"] = new_insts
    return json.dumps(j).encode()


def _build_program(remap):
    key = tuple(int(v) for v in remap)
    if key in _cache:
        return _cache[key]

    import concourse.bass as bass
    import concourse.tile as tile
    from concourse import mybir

    f32 = mybir.dt.float32
    bf16 = mybir.dt.bfloat16
    eff, keeps, groups = _plan(remap)
    NE = len(eff)
    NK = len(keeps)
    kidx = {k: i for i, k in enumerate(keeps)}

    nc = bass.Bass()
    masks_in = nc.dram_tensor("masks", [C, ROWS_PER_CORE, IMG_W], f32,
                              kind="ExternalInput")
    out_dram = nc.dram_tensor("out", [NE, ROWS_PER_CORE, IMG_W], bf16,
                              kind="ExternalOutput")

    with tile.TileContext(nc) as tc:
        with tc.tile_pool(name="main", bufs=NTILES if NTILES > 1 else 1) as pool:
            inp = outp = tree = pool
            plains = [c for c in eff if c not in groups]

            # one-hot source runs: (j0, c0, n).  Group maxes are written
            # in-place over their keep channel in in_tile (the keep's raw
            # value feeds only its own chain), so every source is in_tile
            # and runs are maximal contiguous ranges of eff.
            runs = []
            for j, c in enumerate(eff):
                if runs and runs[-1][1] + runs[-1][2] == c and \
                        runs[-1][0] + runs[-1][2] == j:
                    runs[-1][2] += 1
                else:
                    runs.append([j, c, 1])
            # contiguous runs of keep channels (wide fold pieces)
            keep_runs = []
            for k in keeps:
                if keep_runs and keep_runs[-1][0] + keep_runs[-1][1] == k:
                    keep_runs[-1][1] += 1
                else:
                    keep_runs.append([k, 1])
            for t in range(NTILES):
                sl = slice(G * t, G * (t + 1))
                in_tile = inp.tile([128, C, G], f32, tag="in_tile")
                # channel-group split so the max-folds can start before the
                # whole tile lands; alternate dispatch queues
                cgroups = ([(0, 2), (2, 8), (8, 16), (16, 24), (24, 32)]
                           if t == 0 else [(0, 8), (8, 16), (16, 24), (24, 32)])
                for c0, c1 in cgroups:
                    nc.sync.dma_start(
                        in_tile[:, c0:c1, :],
                        masks_in[c0:c1, :, sl].rearrange(
                            "d p g -> p d g"))

                # group maxes are written in-place over their keep channel
                # (the keep's raw value feeds only its own chain); the global
                # max folds wide runs of finalized channels.
                NS = max(8, C // 2)      # scratch cols for wide folds
                # bufs=1: scratch is written and read only by the DVE
                # (serial), so cross-tile reuse needs no double-buffering
                gs = tree.tile([128, 1 + NS, G], f32, tag="gs", bufs=1)
                acc = gs[:, 0, :]
                MAX = mybir.AluOpType.max

                def TT(out, in0, in1, op):
                    nc.vector.scalar_tensor_tensor(
                        out=out, in0=in0, scalar=0.0, in1=in1,
                        op0=mybir.AluOpType.bypass, op1=op)

                prog = {k: 0 for k in keeps}

                def gchain(k, upto):
                    # advance keep k's in-place max chain for sources < upto
                    srcs = groups[k]
                    while prog[k] < len(srcs):
                        jn = max(prog[k], 1)
                        if srcs[jn] >= upto:
                            return
                        TT(out=in_tile[:, k, :], in0=in_tile[:, k, :],
                           in1=in_tile[:, srcs[jn], :], op=MAX)
                        prog[k] = jn + 1

                def wide_fold(pieces, scr0):
                    # width-matched greedy fold; pieces = [tensor, c0, w,
                    # writable]; returns the final [128, G] max AP
                    scr_cur = scr0
                    if len(pieces) == 1 and not pieces[0][3] and \
                            pieces[0][2] > 1:
                        tn, c0, w, _ = pieces[0]
                        h = w // 2
                        TT(out=gs[:, scr0 + 1:scr0 + 1 + h, :],
                           in0=tn[:, c0:c0 + h, :],
                           in1=tn[:, c0 + w - h:c0 + w, :], op=MAX)
                        pieces = [[gs, scr0 + 1, h, True]]
                        if w & 1:
                            pieces.append([tn, c0 + w - h - 1, 1, False])
                        scr_cur = scr0 + h
                    while len(pieces) > 1:
                        pieces.sort(key=lambda p: -p[2])
                        a = pieces.pop(0)
                        b2 = pieces.pop(0)
                        w = b2[2]
                        if a[3]:
                            dst = [a[0], a[1], w, True]
                            resid = [a[0], a[1] + w, a[2] - w, True]
                        elif b2[3]:
                            dst = [b2[0], b2[1], w, True]
                            resid = [a[0], a[1] + w, a[2] - w, a[3]]
                        else:
                            dst = [gs, 1 + scr_cur, w, True]
                            scr_cur += w
                            assert scr_cur <= NS
                            resid = [a[0], a[1] + w, a[2] - w, a[3]]
                        TT(out=dst[0][:, dst[1]:dst[1] + w, :],
                           in0=a[0][:, a[1]:a[1] + w, :],
                           in1=b2[0][:, b2[1]:b2[1] + w, :], op=MAX)
                        pieces.append(dst)
                        if resid[2] > 0:
                            pieces.append(resid)
                    tn, c0, W, _ = pieces[0]
                    while W > 1:
                        h = W // 2
                        TT(out=tn[:, c0:c0 + h, :],
                           in0=tn[:, c0:c0 + h, :],
                           in1=tn[:, c0 + W - h:c0 + W, :], op=MAX)
                        W = h + (W & 1)
                    return tn[:, c0, :]

                if t == 0:
                    # availability-ordered: fold plains (plus duplicate keep
                    # leads, harmless under max) into acc as groups land,
                    # then wide-fold the finalized keep runs
                    acc_items = []
                    dup_done = set()
                    emitted = 0
                    pend = list(plains)
                    for _, b in cgroups:
                        for k in keeps:
                            if prog[k] == 0 and groups[k][0] < b and \
                                    b <= 8 and groups[k][1] >= b and \
                                    k not in dup_done:
                                acc_items.append(in_tile[:, k, :])
                                dup_done.add(k)
                            gchain(k, b)
                        for c in [c for c in pend if c < b]:
                            acc_items.append(in_tile[:, c, :])
                            pend.remove(c)
                        while acc_items and (len(acc_items) >= 2 or emitted):
                            if not emitted:
                                TT(out=acc, in0=acc_items.pop(0),
                                   in1=acc_items.pop(0), op=MAX)
                            else:
                                TT(out=acc, in0=acc,
                                   in1=acc_items.pop(0), op=MAX)
                            emitted += 1
                    if not emitted and acc_items:
                        it = acc_items.pop(0)
                        TT(out=acc, in0=it, in1=it, op=MAX)
                        emitted = 1
                    pieces = [[in_tile, c0, w, False] for c0, w in keep_runs]
                    if emitted:
                        pieces.append([gs, 0, 1, True])   # acc
                    mx_src = wide_fold(pieces, 0)
                else:
                    # later tiles are never DMA-starved: full chains, then
                    # one wide fold over the contiguous runs of eff
                    for k in keeps:
                        gchain(k, C)
                    pieces = [[in_tile, c0, w, False] for _, c0, w in runs]
                    mx_src = wide_fold(pieces, 0)

                # one-hot over effective channels; pieces DMA'd as soon as
                # their runs are done.  The global max lives in mx_src.
                mx_ap = mx_src
                out_tile = outp.tile([128, NE, G], bf16, tag="out_tile")

                def emit_half(hruns, j0, j1):
                    for r0, c0, n in hruns:
                        mx_b = bass.AP(
                            tensor=mx_ap.tensor, offset=mx_ap.offset,
                            ap=[mx_ap.ap[0], [0, n], mx_ap.ap[-1]])
                        TT(out=out_tile[:, r0:r0 + n, :],
                           in0=in_tile[:, c0:c0 + n, :],
                           in1=mx_b, op=mybir.AluOpType.is_equal)
                    nc.scalar.dma_start(
                        out_dram[j0:j1, :, sl].rearrange("c p g -> p c g"),
                        out_tile[:, j0:j1, :])

                # emit one-hot + out-DMA in ~NE/4-channel pieces so the
                # out stream overlaps the compares and the tail is short
                target = max(2, (NE + 3) // 4)
                bounds, nxt = [], target
                for j0b, _, nb in runs:
                    if j0b + nb >= nxt and j0b + nb < NE:
                        bounds.append(j0b + nb)
                        nxt = j0b + nb + target
                pieces, lo = [], 0
                for bd in bounds + [NE]:
                    pieces.append((lo, bd))
                    lo = bd
                for lo, hi in pieces:
                    emit_half([r for r in runs if lo <= r[0] < hi], lo, hi)

    orig = nc.to_json_bytes
    nc.to_json_bytes = lambda: _split_excess_waits(orig())
    _cache[key] = nc
    return nc


def kernel(masks, slot_features, pad_left, pad_top):
    from concourse.bass_utils import run_bass_kernel_spmd

    masks = np.asarray(masks, np.float32)
    slot_features = np.asarray(slot_features, np.float32)
    pl = [int(v) for v in np.asarray(pad_left)]
    pt = [int(v) for v in np.asarray(pad_top)]

    remap = _compute_remap(masks, slot_features, pl, pt)
    nc = _build_program(remap)

    in_maps = []
    for i in range(N_CORES):
        slab = np.ascontiguousarray(
            masks[0, :, i * ROWS_PER_CORE:(i + 1) * ROWS_PER_CORE, :])
        in_maps.append({"masks": slab})

    res = run_bass_kernel_spmd(nc, in_maps, core_ids=list(range(N_CORES)))

    eff, _, _ = _plan(remap)
    out = np.zeros((1, C, IMG_H, IMG_W), np.float32)
    for i, r in enumerate(res.results):
        out[0, eff, i * ROWS_PER_CORE:(i + 1) * ROWS_PER_CORE, :] = (
            np.asarray(r["out"]).astype(np.float32))

    # patch tied-max pixels (channel-sum != 1) from the input directly
    s = out[0].sum(axis=0)
    ys, xs = np.nonzero(s != 1.0)
    for y, x in zip(ys, xs):
        d = int(np.argmax(masks[0, :, y, x]))
        out[0, :, y, x] = 0.0
        out[0, remap[d], y, x] = 1.0
    return out
